# revision 25
# baseline (speedup 1.0000x reference)
"""Trainium2 Bass kernel for nn_BaselineMemory (sparse attention memory read + MLP).

Data-parallel over batch: each of 8 NeuronCores handles 256 of 2048 rows.
fp16 end-to-end (fp32 PSUM/accum). Host pre-normalizes x and mem^T.
Pipeline per core:
  dist matmul z = x_hat @ y_hat^T (fp16, PE) -> z fp16 + block sums/maxes
  -> sparsemax tau via 3 Newton rounds from a Gaussian-moment init
     (S(tau) = ACT relu head + DVE tail; support count k on DVE is_ge;
      tau += (S-1)/k) -> w materialization pass
  -> w^T transposes (PE, fp16) -> memory read mv^T (fp16, PE)
  -> MLP1 (W1 natural layout, relu+b1 fused into post-transpose evac)
  -> MLP2 (+b2 via rank-1 matmul) -> fp32 out.
DMA: memT stream + memR ring + outputs on the Sync queue; W1/W2 on the
GpSimd queue so they transfer during the sparsemax window.
"""
import sys

if "/opt/trn_rl_repo" not in sys.path:
    sys.path.insert(0, "/opt/trn_rl_repo")

import numpy as np

import concourse.bass as bass  # noqa: F401
import concourse.tile as tile
from concourse import bacc, mybir
from concourse.bass_utils import run_bass_kernel_spmd
from concourse.masks import make_identity

P = 128
B_CORE = 256          # batch rows per core
NBT = B_CORE // P     # 2 b-tiles
D = 1024
DC = D // P           # 8 d-chunks
M = 8192
MB = M // 512         # 16 dist m-blocks per bt
MC = M // P           # 64 m-chunks for read
H = 2048
HC = H // P           # 16 h-chunks
HB = H // 512         # 4 mlp1 col-blocks
OUT = 1000
NOH = 2               # out halves of 500
NW = OUT // NOH

N_ROUNDS = 2          # quasi-Newton iterations on tau
MA = 4608             # S-pass head handled by ACT; tail [MA, M) on DVE
TAIL = M - MA
T0_SIG = 2.25 / 32.0  # init: tau0 = mean + 2.25*sigma, sigma = 1/sqrt(d)
INV_S2 = 22.627417    # 1/(sigma*sqrt(2)) = 32/sqrt(2)
KHALF = 4096.0        # m/2 for the erfc slope model
CLIP = 1.0 / 16.0     # step clip (2*sigma)
CAP_OFF = 1e-4
RING = 24             # memR ring slabs resident
W2BUF = 8
LAG = 12              # read matmuls trail w^T transposes by LAG chunks

F32 = mybir.dt.float32
F16 = mybir.dt.float16
AF = mybir.ActivationFunctionType
ALU = mybir.AluOpType
AX = mybir.AxisListType

_EPS = 1e-6


def build():
    nc = bacc.Bacc("TRN2", target_bir_lowering=False, debug=False)

    xhT_d = nc.dram_tensor("xhT", [P, DC * B_CORE], F16, kind="ExternalInput")
    memT = nc.dram_tensor("memT", [DC, P, M], F16, kind="ExternalInput")
    memR = nc.dram_tensor("memR", [MC, P, D], F16, kind="ExternalInput")
    w1c = nc.dram_tensor("w1c", [DC, P, H], F16, kind="ExternalInput")
    w2c = nc.dram_tensor("w2c", [HC, P, OUT], F16, kind="ExternalInput")
    b1_t = nc.dram_tensor("b1_t", [P, HC], F32, kind="ExternalInput")
    b2_r = nc.dram_tensor("b2_r", [1, OUT], F16, kind="ExternalInput")
    out_d = nc.dram_tensor("out", [NBT, P, OUT], F32, kind="ExternalOutput")

    with tile.TileContext(nc) as tc:
        small = tc.alloc_tile_pool(name="small", bufs=1)
        wpool = tc.alloc_tile_pool(name="wpool", bufs=1)
        w1p = tc.alloc_tile_pool(name="w1p", bufs=1)

        ident = small.tile([P, P], F16, tag="ident")
        make_identity(nc, ident[:])
        ones1 = small.tile([1, P], F16, tag="ones1")
        nc.vector.memset(ones1[:], 1.0)
        b1t = small.tile([P, HC], F32, tag="b1")
        nc.gpsimd.dma_start(b1t[:], b1_t[:])
        b2t = small.tile([1, OUT], F16, tag="b2")
        nc.gpsimd.dma_start(b2t[:], b2_r[:])
        xh = small.tile([P, DC * B_CORE], F16, tag="xh")
        nc.sync.dma_start(xh[:], xhT_d[:])

        w = [wpool.tile([P, M], F16, tag=f"w{bt}", name=f"w{bt}")
             for bt in range(NBT)]
        w1t = [w1p.tile([P, H], F16, tag=f"w1_{dc}", name=f"w1_{dc}")
               for dc in range(DC)]

        st = {}
        for bt in range(NBT):
            d = {}
            d["mx"] = small.tile([P, MB], F32, tag=f"mx{bt}", name=f"mx{bt}")
            d["zsum"] = small.tile([P, MB], F32, tag=f"zs{bt}", name=f"zs{bt}")
            for nm in ["rm", "cap", "zsr", "mu", "sact", "gacc", "targ",
                       "erf", "kg", "rk", "sv", "step", "stepc"]:
                d[nm] = small.tile([P, 1], F32, tag=f"{nm}{bt}", name=f"{nm}{bt}")
            d["tau"] = [small.tile([P, 1], F32, tag=f"tau{bt}_{r}",
                                   name=f"tau{bt}_{r}")
                        for r in range(N_ROUNDS + 1)]
            d["ntau"] = [small.tile([P, 1], F32, tag=f"ntau{bt}_{r}",
                                    name=f"ntau{bt}_{r}")
                         for r in range(N_ROUNDS + 1)]
            st[bt] = d

        # ---- persistent row tiles (released after w materialization) ----
        zpool = tc.alloc_tile_pool(name="zpool", bufs=1)
        z = [zpool.tile([P, M], F16, tag=f"z{bt}", name=f"z{bt}")
             for bt in range(NBT)]
        scr = [zpool.tile([P, TAIL], F16, tag=f"scr{bt}", name=f"scr{bt}")
               for bt in range(NBT)]
        tr1 = [zpool.tile([P, TAIL // 2], F16, tag=f"tr1{bt}", name=f"tr1{bt}")
               for bt in range(NBT)]
        tr2 = [zpool.tile([P, TAIL // 4], F16, tag=f"tr2{bt}", name=f"tr2{bt}")
               for bt in range(NBT)]
        tr3 = [zpool.tile([P, TAIL // 8], F16, tag=f"tr3{bt}", name=f"tr3{bt}")
               for bt in range(NBT)]

        # ---- PE warmup burst: ramp the clock while the first DMAs land ----
        junk = small.tile([P, 512], F16, tag="junk")
        nc.vector.memset(junk[:], 1.0)

        # ---- dist: z[bt] [P, M] fp16 + block sums/maxes ----
        mstream = tc.alloc_tile_pool(name="mstream", bufs=2)
        ps_wu = tc.alloc_tile_pool(name="ps_wu", bufs=2, space="PSUM")
        for i in range(10):
            wup = ps_wu.tile([P, 512], F32, tag="wu")
            nc.tensor.matmul(wup[:], ident[:], junk[:], start=True, stop=True)
        ps_wu.release()
        ps_dist = tc.alloc_tile_pool(name="ps_dist", bufs=4, space="PSUM")
        ps_warm = tc.alloc_tile_pool(name="ps_warm", bufs=2, space="PSUM")
        NSTAT = 12            # half-blocks used for the tau0 stats

        def emit_init():
            for bt in range(NBT):
                d = st[bt]
                nc.vector.reduce_sum(d["zsr"][:], d["zsum"][:, 0:NSTAT],
                                     axis=AX.X)
                nc.vector.reduce_max(d["rm"][:], d["mx"][:, 0:NSTAT],
                                     axis=AX.X)
                nc.vector.tensor_scalar_add(d["cap"][:], d["rm"][:], -CAP_OFF)
                nc.vector.tensor_scalar_mul(d["mu"][:], d["zsr"][:],
                                            1.0 / (NSTAT * 512))
                nc.vector.tensor_scalar_add(d["step"][:], d["mu"][:], T0_SIG)
                nc.vector.tensor_tensor(
                    d["tau"][0][:], d["step"][:], d["cap"][:], ALU.min)
                nc.vector.tensor_scalar_mul(d["ntau"][0][:], d["tau"][0][:],
                                            -1.0)

        def emit_round_head(r):
            for bt in range(NBT):
                d = st[bt]
                nc.vector.tensor_scalar(
                    out=d["targ"][:], in0=d["tau"][r][:],
                    scalar1=d["mu"][:, 0:1],
                    scalar2=INV_S2, op0=ALU.subtract, op1=ALU.mult)
            for bt in range(NBT):
                d = st[bt]
                nc.scalar.activation(
                    w[bt][:, 0:MA], z[bt][:, 0:MA], AF.Relu,
                    bias=d["ntau"][r][:, 0:1], accum_out=d["sact"][:])
                nc.scalar.activation(d["erf"][:], d["targ"][:], AF.Erf)
            for bt in range(NBT):
                d = st[bt]
                nc.vector.tensor_scalar(
                    out=d["kg"][:], in0=d["erf"][:], scalar1=-KHALF,
                    scalar2=KHALF, op0=ALU.mult, op1=ALU.add)
                nc.vector.tensor_scalar_max(d["kg"][:], d["kg"][:], 1.0)
                nc.vector.reciprocal(d["rk"][:], d["kg"][:])

        def emit_round_tail(r):
            for bt in range(NBT):
                d = st[bt]
                tau_s = d["tau"][r][:, 0:1]
                nc.vector.tensor_scalar(
                    out=scr[bt][:], in0=z[bt][:, MA:M],
                    scalar1=tau_s, scalar2=None, op0=ALU.subtract)
                nc.vector.tensor_scalar(
                    out=w[bt][:, MA:M], in0=scr[bt][:],
                    scalar1=0.0, scalar2=None, op0=ALU.max)
                h2 = TAIL // 2
                h4 = TAIL // 4
                nc.vector.tensor_add(
                    tr1[bt][:], w[bt][:, MA:MA + h2], w[bt][:, MA + h2:M])
                nc.vector.tensor_add(
                    tr2[bt][:], tr1[bt][:, 0:h4], tr1[bt][:, h4:h2])
                h8 = TAIL // 8
                nc.vector.tensor_add(
                    tr3[bt][:], tr2[bt][:, 0:h8], tr2[bt][:, h8:h4])
                nc.vector.tensor_scalar(
                    out=tr3[bt][:], in0=tr3[bt][:], scalar1=0.0, scalar2=None,
                    op0=ALU.add, op1=ALU.add, accum_out=d["gacc"][:])
                nc.vector.tensor_add(d["sv"][:], d["sact"][:], d["gacc"][:])
                nc.vector.tensor_scalar(
                    out=d["step"][:], in0=d["sv"][:], scalar1=-1.0,
                    scalar2=d["rk"][:, 0:1], op0=ALU.add, op1=ALU.mult)
                nc.vector.tensor_scalar(
                    out=d["stepc"][:], in0=d["step"][:], scalar1=CLIP,
                    scalar2=-CLIP, op0=ALU.min, op1=ALU.max)
                nc.vector.tensor_scalar(
                    out=d["tau"][r + 1][:], in0=d["stepc"][:],
                    scalar1=d["tau"][r][:, 0:1], scalar2=d["cap"][:, 0:1],
                    op0=ALU.add, op1=ALU.min)
                nc.vector.tensor_scalar_mul(
                    d["ntau"][r + 1][:], d["tau"][r + 1][:], -1.0)
            # keep the PE clock from dropping to the lowest p-state
            for i in range(4):
                wp = ps_warm.tile([P, 512], F32, tag="warm")
                nc.tensor.matmul(wp[:], ident[:],
                                 scr[i % NBT][:, 0:512], start=True, stop=True)

        for blk in range(MB // 2):
            mt = mstream.tile([P, DC, 1024], F16, tag="memT")
            if blk == 0:
                # split block 0 by m-halves: the first matmuls start as soon
                # as the first half lands
                for mh in range(2):
                    nc.sync.dma_start(
                        mt[:, :, mh * 512:(mh + 1) * 512],
                        memT[:, :, mh * 512:(mh + 1) * 512]
                        .rearrange("d p m -> p d m"))
            else:
                for dq in range(2):
                    nc.sync.dma_start(
                        mt[:, dq * 4:(dq + 1) * 4],
                        memT[dq * 4:(dq + 1) * 4,
                             :, blk * 1024:(blk + 1) * 1024]
                        .rearrange("d p m -> p d m"))
            for mh in range(2):
                mb = blk * 2 + mh
                for bt in range(NBT):
                    zp = ps_dist.tile([P, 512], F32, tag="zp")
                    for dc in range(DC):
                        nc.tensor.matmul(
                            zp[:],
                            xh[:, dc * B_CORE + bt * P:
                               dc * B_CORE + (bt + 1) * P],
                            mt[:, dc, mh * 512:(mh + 1) * 512],
                            start=(dc == 0), stop=(dc == DC - 1))
                    if mb < NSTAT:
                        nc.scalar.activation(
                            z[bt][:, mb * 512:(mb + 1) * 512], zp[:], AF.Copy,
                            accum_out=st[bt]["zsum"][:, mb:mb + 1])
                        nc.vector.reduce_max(
                            st[bt]["mx"][:, mb:mb + 1],
                            z[bt][:, mb * 512:(mb + 1) * 512], axis=AX.X)
                    else:
                        # late blocks evacuate on DVE so the ACT queue is
                        # free for round-0 heads during the dist tail
                        nc.vector.tensor_copy(
                            z[bt][:, mb * 512:(mb + 1) * 512], zp[:])
            if blk == (NSTAT // 2) - 1:
                emit_init()
                emit_round_head(0)



        # ---- remaining rounds (round-0 head already ran inside dist) ----
        emit_round_tail(0)
        for r in range(1, N_ROUNDS):
            emit_round_head(r)
            emit_round_tail(r)

        # ---- final w materialization at converged tau (split into column
        # halves so the first w^T transposes can start sooner) ----
        MAH = MA // 2
        for bt in range(NBT):
            d = st[bt]
            nf = d["ntau"][N_ROUNDS]
            nc.scalar.activation(
                w[bt][:, 0:MAH], z[bt][:, 0:MAH], AF.Relu, bias=nf[:, 0:1])
        for bt in range(NBT):
            d = st[bt]
            nf = d["ntau"][N_ROUNDS]
            nc.scalar.activation(
                w[bt][:, MAH:MA], z[bt][:, MAH:MA], AF.Relu, bias=nf[:, 0:1])
        for bt in range(NBT):
            d = st[bt]
            tau_s = d["tau"][N_ROUNDS][:, 0:1]
            nc.vector.tensor_scalar(
                out=scr[bt][:], in0=z[bt][:, MA:M],
                scalar1=tau_s, scalar2=None, op0=ALU.subtract)
            nc.vector.tensor_scalar(
                out=w[bt][:, MA:M], in0=scr[bt][:],
                scalar1=0.0, scalar2=None, op0=ALU.max)
        for i in range(4):
            wp = ps_warm.tile([P, 512], F32, tag="warm")
            nc.tensor.matmul(wp[:], ident[:], junk[:], start=True, stop=True)
        ps_warm.release()
        ps_dist.release()
        mstream.release()
        zpool.release()

        # ---- w^T transposes + memory read: mv[bt] = w[bt] @ memR ----
        wTt = tc.alloc_tile_pool(name="wTt", bufs=16)
        mring = tc.alloc_tile_pool(name="mring", bufs=3)
        w2s = tc.alloc_tile_pool(name="w2s", bufs=W2BUF)
        ps_tr = tc.alloc_tile_pool(name="ps_tr", bufs=4, space="PSUM")
        ps_mv = tc.alloc_tile_pool(name="ps_mv", bufs=1, space="PSUM")

        # memR ring: 8-slab groups (big DMAs amortize the trigger cost).
        # Prefill transfers run during the sparsemax window.
        GRP = 8
        NGRP = MC // GRP
        PRE = 3
        mgrp = []
        for g in range(NGRP):
            gt = mring.tile([P, GRP, D], F16, tag="memR", name=f"memR{g}")
            mgrp.append(gt)
            if g < PRE:
                nc.sync.dma_start(
                    gt[:], memR[g * GRP:(g + 1) * GRP]
                    .rearrange("c p d -> p c d"))
        # weights after the prefill: they arrive during the sparsemax window
        for dc in range(DC):
            nc.sync.dma_start(w1t[dc][:], w1c[dc])
        w2t = [w2s.tile([P, 2, OUT], F16, tag="w2", name=f"w2p_{g}")
               for g in range(HC // 2)]
        for g in range(HC // 2):
            nc.sync.dma_start(
                w2t[g][:],
                w2c[g * 2:(g + 1) * 2].rearrange("c p o -> p c o"))

        mv_ps = [[ps_mv.tile([P, 512], F32, tag=f"mv{bt}_{dh}",
                             name=f"mv{bt}_{dh}")
                  for dh in range(2)] for bt in range(NBT)]
        # transposes run LAG chunks ahead of the read matmuls so the PE never
        # stalls on the cross-engine psum->sbuf evacuation roundtrip
        wTs = []
        for it in range(MC + LAG):
            if it < MC:
                mc = it
                if mc % GRP == 0 and mc >= PRE * GRP:
                    g = mc // GRP
                    nc.sync.dma_start(
                        mgrp[g][:], memR[g * GRP:(g + 1) * GRP]
                        .rearrange("c p d -> p c d"))
                tp = ps_tr.tile([P, B_CORE], F16, tag="wtr")
                for bt in range(NBT):
                    nc.tensor.transpose(
                        tp[:, bt * P:(bt + 1) * P],
                        w[bt][:, mc * P:(mc + 1) * P], ident[:])
                wT = wTt.tile([P, B_CORE], F16, tag="wT", name=f"wT{mc}")
                wTs.append(wT)
                if mc % 2 == 0:
                    nc.vector.tensor_copy(wT[:], tp[:])
                else:
                    nc.scalar.copy(wT[:], tp[:])
            if it >= LAG:
                mc = it - LAG
                gt = mgrp[mc // GRP]
                sl = mc % GRP
                for bt in range(NBT):
                    for dh in range(2):
                        nc.tensor.matmul(
                            mv_ps[bt][dh][:], wTs[mc][:, bt * P:(bt + 1) * P],
                            gt[:, sl, dh * 512:(dh + 1) * 512],
                            start=(mc == 0), stop=(mc == MC - 1))

        # ---- mv evac (fp16) + transpose to mvT [P, dc, 256] ----
        mv_sb = [small.tile([P, D], F16, tag=f"mvsb{bt}", name=f"mvsb{bt}")
                 for bt in range(NBT)]
        for bt in range(NBT):
            for dh in range(2):
                nc.scalar.copy(mv_sb[bt][:, dh * 512:(dh + 1) * 512],
                               mv_ps[bt][dh][:])
        ps_mv.release()
        mvT = small.tile([P, DC, B_CORE], F16, tag="mvT")
        for dc in range(DC):
            tp = ps_tr.tile([P, B_CORE], F16, tag="wtr")
            for bt in range(NBT):
                nc.tensor.transpose(
                    tp[:, bt * P:(bt + 1) * P],
                    mv_sb[bt][:, dc * P:(dc + 1) * P], ident[:])
            if dc % 2 == 0:
                nc.vector.tensor_copy(mvT[:, dc], tp[:])
            else:
                nc.scalar.copy(mvT[:, dc], tp[:])

        # ---- MLP1: h[bt] [P(b), H] = mvT-blocks^T @ W1-chunks (bias later) --
        hsb = [small.tile([P, H], F16, tag=f"h{bt}", name=f"h{bt}")
               for bt in range(NBT)]
        ps_h = tc.alloc_tile_pool(name="ps_h", bufs=1, space="PSUM")
        hps = [ps_h.tile([P, 512], F32, tag=f"hp{hb}", name=f"hp{hb}")
               for hb in range(HB)]
        for bt in range(NBT):
            for dc in range(DC):
                for hb in range(HB):
                    nc.tensor.matmul(
                        hps[hb][:], mvT[:, dc, bt * P:(bt + 1) * P],
                        w1t[dc][:, hb * 512:(hb + 1) * 512],
                        start=(dc == 0), stop=(dc == DC - 1))
            for hb in range(HB):
                nc.scalar.copy(hsb[bt][:, hb * 512:(hb + 1) * 512],
                               hps[hb][:])
        ps_h.release()

        # ---- hT transposes; relu + b1 fused into the per-partition evac ----
        hT = small.tile([P, HC, B_CORE], F16, tag="hT")
        for hc in range(HC):
            tp = ps_tr.tile([P, B_CORE], F16, tag="wtr")
            for bt in range(NBT):
                nc.tensor.transpose(
                    tp[:, bt * P:(bt + 1) * P],
                    hsb[bt][:, hc * P:(hc + 1) * P], ident[:])
            nc.scalar.activation(
                hT[:, hc], tp[:], AF.Relu, bias=b1t[:, hc:hc + 1])

        # ---- MLP2: out[bt] = hT-blocks^T @ W2 + b2 ----
        ps_o = tc.alloc_tile_pool(name="ps_o", bufs=1, space="PSUM")
        osb = [small.tile([P, OUT], F32, tag=f"osb{bt}", name=f"osb{bt}")
               for bt in range(NBT)]
        ops = [[ps_o.tile([P, NW], F32, tag=f"op{bt}_{oh}",
                          name=f"op{bt}_{oh}")
                for oh in range(NOH)] for bt in range(NBT)]
        for bt in range(NBT):
            for oh in range(NOH):
                nc.tensor.matmul(
                    ops[bt][oh][:], ones1[:], b2t[:, oh * NW:(oh + 1) * NW],
                    start=True, stop=False)
        for kc in range(HC):
            for bt in range(NBT):
                for oh in range(NOH):
                    nc.tensor.matmul(
                        ops[bt][oh][:], hT[:, kc, bt * P:(bt + 1) * P],
                        w2t[kc // 2][:, kc % 2, oh * NW:(oh + 1) * NW],
                        start=False, stop=(kc == HC - 1))
        for bt in range(NBT):
            for oh in range(NOH):
                nc.scalar.copy(osb[bt][:, oh * NW:(oh + 1) * NW],
                               ops[bt][oh][:])
                nc.gpsimd.dma_start(
                    out_d[bt, :, oh * NW:(oh + 1) * NW],
                    osb[bt][:, oh * NW:(oh + 1) * NW])
        ps_o.release()
        ps_tr.release()
        w2s.release()
        mring.release()
        wTt.release()
        w1p.release()
        wpool.release()
        small.release()

    nc.compile()
    return nc


_CACHED = None


def _prep(inputs):
    x = np.ascontiguousarray(inputs["encoder_output"], dtype=np.float32)
    mem = np.ascontiguousarray(inputs["memory_set"], dtype=np.float32)
    W1 = np.ascontiguousarray(inputs["W1"], dtype=np.float32)
    b1 = np.ascontiguousarray(inputs["b1"], dtype=np.float32)
    W2 = np.ascontiguousarray(inputs["W2"], dtype=np.float32)
    b2 = np.ascontiguousarray(inputs["b2"], dtype=np.float32)

    inv_nx = 1.0 / np.sqrt((x * x).sum(1) + _EPS)
    inv_ny = 1.0 / np.sqrt((mem * mem).sum(1) + _EPS)
    xh = (x * inv_nx[:, None]).astype(np.float16)
    memT_hat = np.ascontiguousarray(
        (mem.T * inv_ny[None, :]).astype(np.float16).reshape(DC, P, M))
    memR_v = np.ascontiguousarray(mem.astype(np.float16).reshape(MC, P, D))
    w1_blk = np.ascontiguousarray(W1.astype(np.float16).reshape(DC, P, H))
    w2_blk = np.ascontiguousarray(W2.astype(np.float16).reshape(HC, P, OUT))
    b1_tiles = np.ascontiguousarray(b1.reshape(HC, P).T.astype(np.float32))
    b2_row = np.ascontiguousarray(b2.reshape(1, OUT).astype(np.float16))

    shared = {
        "memT": memT_hat, "memR": memR_v, "w1c": w1_blk,
        "w2c": w2_blk, "b1_t": b1_tiles, "b2_r": b2_row,
    }
    in_maps = []
    for c in range(8):
        xc = xh[c * B_CORE:(c + 1) * B_CORE]          # [256, 1024]
        xhT = np.ascontiguousarray(
            xc.T.reshape(DC, P, B_CORE).transpose(1, 0, 2)
            .reshape(P, DC * B_CORE))
        in_maps.append({"xhT": xhT, **shared})
    return in_maps


def kernel(**inputs) -> np.ndarray:
    global _CACHED
    if _CACHED is None:
        _CACHED = build()
    nc = _CACHED
    in_maps = _prep(inputs)
    res = run_bass_kernel_spmd(nc, in_maps, core_ids=list(range(8)))
    return np.concatenate(
        [r["out"].reshape(B_CORE, OUT) for r in res.results], axis=0)


# revision 26
# speedup vs baseline: 1.0606x; 1.0606x over previous
"""Trainium2 Bass kernel for nn_BaselineMemory (sparse attention memory read + MLP).

Data-parallel over batch: each of 8 NeuronCores handles 256 of 2048 rows.
fp16 end-to-end (fp32 PSUM/accum). Host pre-normalizes x and mem^T.
Pipeline per core:
  dist matmul z = x_hat @ y_hat^T (fp16, PE) -> z fp16 + block sums/maxes
  -> sparsemax tau via 3 Newton rounds from a Gaussian-moment init
     (S(tau) = ACT relu head + DVE tail; support count k on DVE is_ge;
      tau += (S-1)/k) -> w materialization pass
  -> w^T transposes (PE, fp16) -> memory read mv^T (fp16, PE)
  -> MLP1 (W1 natural layout, relu+b1 fused into post-transpose evac)
  -> MLP2 (+b2 via rank-1 matmul) -> fp32 out.
DMA: memT stream + memR ring + outputs on the Sync queue; W1/W2 on the
GpSimd queue so they transfer during the sparsemax window.
"""
import sys

if "/opt/trn_rl_repo" not in sys.path:
    sys.path.insert(0, "/opt/trn_rl_repo")

import numpy as np

import concourse.bass as bass  # noqa: F401
import concourse.tile as tile
from concourse import bacc, mybir
from concourse.bass_utils import run_bass_kernel_spmd
from concourse.masks import make_identity

P = 128
B_CORE = 256          # batch rows per core
NBT = B_CORE // P     # 2 b-tiles
D = 1024
DC = D // P           # 8 d-chunks
M = 8192
MB = M // 512         # 16 dist m-blocks per bt
MC = M // P           # 64 m-chunks for read
H = 2048
HC = H // P           # 16 h-chunks
HB = H // 512         # 4 mlp1 col-blocks
OUT = 1000
NOH = 2               # out halves of 500
NW = OUT // NOH

N_ROUNDS = 2          # quasi-Newton iterations on tau
MA = 5120             # S-pass head handled by ACT; tail [MA, M) on DVE
TAIL = M - MA
T0_SIG = 2.25 / 32.0  # init: tau0 = mean + 2.25*sigma, sigma = 1/sqrt(d)
INV_S2 = 22.627417    # 1/(sigma*sqrt(2)) = 32/sqrt(2)
KHALF = 4096.0        # m/2 for the erfc slope model
CLIP = 1.0 / 16.0     # step clip (2*sigma)
CAP_OFF = 1e-4
RING = 24             # memR ring slabs resident
W2BUF = 8
LAG = 12              # read matmuls trail w^T transposes by LAG chunks

F32 = mybir.dt.float32
F16 = mybir.dt.float16
AF = mybir.ActivationFunctionType
ALU = mybir.AluOpType
AX = mybir.AxisListType

_EPS = 1e-6


def build():
    nc = bacc.Bacc("TRN2", target_bir_lowering=False, debug=False)

    xhT_d = nc.dram_tensor("xhT", [P, DC * B_CORE], F16, kind="ExternalInput")
    memT = nc.dram_tensor("memT", [DC, P, M], F16, kind="ExternalInput")
    memR = nc.dram_tensor("memR", [MC, P, D], F16, kind="ExternalInput")
    w1c = nc.dram_tensor("w1c", [DC, P, H], F16, kind="ExternalInput")
    w2c = nc.dram_tensor("w2c", [HC, P, OUT], F16, kind="ExternalInput")
    b1_t = nc.dram_tensor("b1_t", [P, HC], F32, kind="ExternalInput")
    b2_r = nc.dram_tensor("b2_r", [1, OUT], F16, kind="ExternalInput")
    out_d = nc.dram_tensor("out", [NBT, P, OUT], F32, kind="ExternalOutput")

    with tile.TileContext(nc) as tc:
        small = tc.alloc_tile_pool(name="small", bufs=1)
        wpool = tc.alloc_tile_pool(name="wpool", bufs=1)
        w1p = tc.alloc_tile_pool(name="w1p", bufs=1)

        ident = small.tile([P, P], F16, tag="ident")
        make_identity(nc, ident[:])
        ones1 = small.tile([1, P], F16, tag="ones1")
        nc.vector.memset(ones1[:], 1.0)
        b1t = small.tile([P, HC], F32, tag="b1")
        nc.gpsimd.dma_start(b1t[:], b1_t[:])
        b2t = small.tile([1, OUT], F16, tag="b2")
        nc.gpsimd.dma_start(b2t[:], b2_r[:])
        xh = small.tile([P, DC * B_CORE], F16, tag="xh")
        nc.sync.dma_start(xh[:], xhT_d[:])

        w = [wpool.tile([P, M], F16, tag=f"w{bt}", name=f"w{bt}")
             for bt in range(NBT)]
        w1t = [w1p.tile([P, H], F16, tag=f"w1_{dc}", name=f"w1_{dc}")
               for dc in range(DC)]

        st = {}
        for bt in range(NBT):
            d = {}
            d["mx"] = small.tile([P, MB], F32, tag=f"mx{bt}", name=f"mx{bt}")
            d["zsum"] = small.tile([P, MB], F32, tag=f"zs{bt}", name=f"zs{bt}")
            for nm in ["rm", "cap", "zsr", "mu", "sact", "gacc", "targ",
                       "erf", "kg", "rk", "sv", "step", "stepc"]:
                d[nm] = small.tile([P, 1], F32, tag=f"{nm}{bt}", name=f"{nm}{bt}")
            d["tau"] = [small.tile([P, 1], F32, tag=f"tau{bt}_{r}",
                                   name=f"tau{bt}_{r}")
                        for r in range(N_ROUNDS + 1)]
            d["ntau"] = [small.tile([P, 1], F32, tag=f"ntau{bt}_{r}",
                                    name=f"ntau{bt}_{r}")
                         for r in range(N_ROUNDS + 1)]
            st[bt] = d

        # ---- persistent row tiles (released after w materialization) ----
        zpool = tc.alloc_tile_pool(name="zpool", bufs=1)
        z = [zpool.tile([P, M], F16, tag=f"z{bt}", name=f"z{bt}")
             for bt in range(NBT)]
        scr = [zpool.tile([P, TAIL], F16, tag=f"scr{bt}", name=f"scr{bt}")
               for bt in range(NBT)]
        tr1 = [zpool.tile([P, TAIL // 2], F16, tag=f"tr1{bt}", name=f"tr1{bt}")
               for bt in range(NBT)]
        tr2 = [zpool.tile([P, TAIL // 4], F16, tag=f"tr2{bt}", name=f"tr2{bt}")
               for bt in range(NBT)]
        tr3 = [zpool.tile([P, TAIL // 8], F16, tag=f"tr3{bt}", name=f"tr3{bt}")
               for bt in range(NBT)]

        # ---- PE warmup burst: ramp the clock while the first DMAs land ----
        junk = small.tile([P, 512], F16, tag="junk")
        nc.vector.memset(junk[:], 1.0)

        # ---- dist: z[bt] [P, M] fp16 + block sums/maxes ----
        mstream = tc.alloc_tile_pool(name="mstream", bufs=2)
        ps_wu = tc.alloc_tile_pool(name="ps_wu", bufs=2, space="PSUM")
        for i in range(10):
            wup = ps_wu.tile([P, 512], F32, tag="wu")
            nc.tensor.matmul(wup[:], ident[:], junk[:], start=True, stop=True)
        ps_wu.release()
        ps_dist = tc.alloc_tile_pool(name="ps_dist", bufs=4, space="PSUM")
        ps_warm = tc.alloc_tile_pool(name="ps_warm", bufs=2, space="PSUM")
        NSTAT = 12            # half-blocks used for the tau0 stats

        def emit_init():
            for bt in range(NBT):
                d = st[bt]
                nc.vector.reduce_sum(d["zsr"][:], d["zsum"][:, 0:NSTAT],
                                     axis=AX.X)
                nc.vector.reduce_max(d["rm"][:], d["mx"][:, 0:NSTAT],
                                     axis=AX.X)
                nc.vector.tensor_scalar_add(d["cap"][:], d["rm"][:], -CAP_OFF)
                nc.vector.tensor_scalar_mul(d["mu"][:], d["zsr"][:],
                                            1.0 / (NSTAT * 512))
                nc.vector.tensor_scalar_add(d["step"][:], d["mu"][:], T0_SIG)
                nc.vector.tensor_tensor(
                    d["tau"][0][:], d["step"][:], d["cap"][:], ALU.min)
                nc.vector.tensor_scalar_mul(d["ntau"][0][:], d["tau"][0][:],
                                            -1.0)

        def emit_round_head(r):
            for bt in range(NBT):
                d = st[bt]
                nc.vector.tensor_scalar(
                    out=d["targ"][:], in0=d["tau"][r][:],
                    scalar1=d["mu"][:, 0:1],
                    scalar2=INV_S2, op0=ALU.subtract, op1=ALU.mult)
            for bt in range(NBT):
                d = st[bt]
                nc.scalar.activation(
                    w[bt][:, 0:MA], z[bt][:, 0:MA], AF.Relu,
                    bias=d["ntau"][r][:, 0:1], accum_out=d["sact"][:])
                nc.scalar.activation(d["erf"][:], d["targ"][:], AF.Erf)
            for bt in range(NBT):
                d = st[bt]
                nc.vector.tensor_scalar(
                    out=d["kg"][:], in0=d["erf"][:], scalar1=-KHALF,
                    scalar2=KHALF, op0=ALU.mult, op1=ALU.add)
                nc.vector.tensor_scalar_max(d["kg"][:], d["kg"][:], 1.0)
                nc.vector.reciprocal(d["rk"][:], d["kg"][:])

        def emit_round_tail(r):
            for bt in range(NBT):
                d = st[bt]
                tau_s = d["tau"][r][:, 0:1]
                nc.vector.tensor_scalar(
                    out=scr[bt][:], in0=z[bt][:, MA:M],
                    scalar1=tau_s, scalar2=None, op0=ALU.subtract)
                nc.vector.tensor_scalar(
                    out=w[bt][:, MA:M], in0=scr[bt][:],
                    scalar1=0.0, scalar2=None, op0=ALU.max)
                h2 = TAIL // 2
                h4 = TAIL // 4
                nc.vector.tensor_add(
                    tr1[bt][:], w[bt][:, MA:MA + h2], w[bt][:, MA + h2:M])
                nc.vector.tensor_add(
                    tr2[bt][:], tr1[bt][:, 0:h4], tr1[bt][:, h4:h2])
                h8 = TAIL // 8
                nc.vector.tensor_add(
                    tr3[bt][:], tr2[bt][:, 0:h8], tr2[bt][:, h8:h4])
                nc.vector.tensor_scalar(
                    out=tr3[bt][:], in0=tr3[bt][:], scalar1=0.0, scalar2=None,
                    op0=ALU.add, op1=ALU.add, accum_out=d["gacc"][:])
                nc.vector.tensor_add(d["sv"][:], d["sact"][:], d["gacc"][:])
                nc.vector.tensor_scalar(
                    out=d["step"][:], in0=d["sv"][:], scalar1=-1.0,
                    scalar2=d["rk"][:, 0:1], op0=ALU.add, op1=ALU.mult)
                nc.vector.tensor_scalar(
                    out=d["stepc"][:], in0=d["step"][:], scalar1=CLIP,
                    scalar2=-CLIP, op0=ALU.min, op1=ALU.max)
                nc.vector.tensor_scalar(
                    out=d["tau"][r + 1][:], in0=d["stepc"][:],
                    scalar1=d["tau"][r][:, 0:1], scalar2=d["cap"][:, 0:1],
                    op0=ALU.add, op1=ALU.min)
                nc.vector.tensor_scalar_mul(
                    d["ntau"][r + 1][:], d["tau"][r + 1][:], -1.0)
            # keep the PE clock from dropping to the lowest p-state
            for i in range(4):
                wp = ps_warm.tile([P, 512], F32, tag="warm")
                nc.tensor.matmul(wp[:], ident[:],
                                 scr[i % NBT][:, 0:512], start=True, stop=True)

        for blk in range(MB // 2):
            mt = mstream.tile([P, DC, 1024], F16, tag="memT")
            if blk == 0:
                # split block 0 by m-halves: the first matmuls start as soon
                # as the first half lands
                for mh in range(2):
                    nc.sync.dma_start(
                        mt[:, :, mh * 512:(mh + 1) * 512],
                        memT[:, :, mh * 512:(mh + 1) * 512]
                        .rearrange("d p m -> p d m"))
            else:
                for dq in range(2):
                    nc.sync.dma_start(
                        mt[:, dq * 4:(dq + 1) * 4],
                        memT[dq * 4:(dq + 1) * 4,
                             :, blk * 1024:(blk + 1) * 1024]
                        .rearrange("d p m -> p d m"))
            for mh in range(2):
                mb = blk * 2 + mh
                for bt in range(NBT):
                    zp = ps_dist.tile([P, 512], F32, tag="zp")
                    for dc in range(DC):
                        nc.tensor.matmul(
                            zp[:],
                            xh[:, dc * B_CORE + bt * P:
                               dc * B_CORE + (bt + 1) * P],
                            mt[:, dc, mh * 512:(mh + 1) * 512],
                            start=(dc == 0), stop=(dc == DC - 1))
                    if mb < NSTAT:
                        nc.scalar.activation(
                            z[bt][:, mb * 512:(mb + 1) * 512], zp[:], AF.Copy,
                            accum_out=st[bt]["zsum"][:, mb:mb + 1])
                        nc.vector.reduce_max(
                            st[bt]["mx"][:, mb:mb + 1],
                            z[bt][:, mb * 512:(mb + 1) * 512], axis=AX.X)
                    else:
                        # late blocks evacuate on DVE so the ACT queue is
                        # free for round-0 heads during the dist tail
                        nc.vector.tensor_copy(
                            z[bt][:, mb * 512:(mb + 1) * 512], zp[:])
            if blk == (NSTAT // 2) - 1:
                emit_init()
                emit_round_head(0)



        # ---- remaining rounds (round-0 head already ran inside dist) ----
        emit_round_tail(0)
        for r in range(1, N_ROUNDS):
            emit_round_head(r)
            emit_round_tail(r)

        # ---- final w materialization at converged tau (split into column
        # halves so the first w^T transposes can start sooner) ----
        MAH = MA // 2
        for bt in range(NBT):
            d = st[bt]
            nf = d["ntau"][N_ROUNDS]
            nc.scalar.activation(
                w[bt][:, 0:MAH], z[bt][:, 0:MAH], AF.Relu, bias=nf[:, 0:1])
        for bt in range(NBT):
            d = st[bt]
            nf = d["ntau"][N_ROUNDS]
            nc.scalar.activation(
                w[bt][:, MAH:MA], z[bt][:, MAH:MA], AF.Relu, bias=nf[:, 0:1])
        for bt in range(NBT):
            d = st[bt]
            tau_s = d["tau"][N_ROUNDS][:, 0:1]
            nc.vector.tensor_scalar(
                out=scr[bt][:], in0=z[bt][:, MA:M],
                scalar1=tau_s, scalar2=None, op0=ALU.subtract)
            nc.vector.tensor_scalar(
                out=w[bt][:, MA:M], in0=scr[bt][:],
                scalar1=0.0, scalar2=None, op0=ALU.max)
        for i in range(4):
            wp = ps_warm.tile([P, 512], F32, tag="warm")
            nc.tensor.matmul(wp[:], ident[:], junk[:], start=True, stop=True)
        ps_warm.release()
        ps_dist.release()
        mstream.release()
        zpool.release()

        # ---- w^T transposes + memory read: mv[bt] = w[bt] @ memR ----
        wTt = tc.alloc_tile_pool(name="wTt", bufs=16)
        mring = tc.alloc_tile_pool(name="mring", bufs=3)
        w2s = tc.alloc_tile_pool(name="w2s", bufs=W2BUF)
        ps_tr = tc.alloc_tile_pool(name="ps_tr", bufs=4, space="PSUM")
        ps_mv = tc.alloc_tile_pool(name="ps_mv", bufs=1, space="PSUM")

        # memR ring: 8-slab groups (big DMAs amortize the trigger cost).
        # Prefill transfers run during the sparsemax window.
        GRP = 8
        NGRP = MC // GRP
        PRE = 3
        mgrp = []
        for g in range(NGRP):
            gt = mring.tile([P, GRP, D], F16, tag="memR", name=f"memR{g}")
            mgrp.append(gt)
            if g < PRE:
                nc.sync.dma_start(
                    gt[:], memR[g * GRP:(g + 1) * GRP]
                    .rearrange("c p d -> p c d"))
        # weights after the prefill: they arrive during the sparsemax window
        for dc in range(DC):
            nc.sync.dma_start(w1t[dc][:], w1c[dc])
        w2t = [w2s.tile([P, 2, OUT], F16, tag="w2", name=f"w2p_{g}")
               for g in range(HC // 2)]
        for g in range(HC // 2):
            nc.sync.dma_start(
                w2t[g][:],
                w2c[g * 2:(g + 1) * 2].rearrange("c p o -> p c o"))

        mv_ps = [[ps_mv.tile([P, 512], F32, tag=f"mv{bt}_{dh}",
                             name=f"mv{bt}_{dh}")
                  for dh in range(2)] for bt in range(NBT)]
        # transposes run LAG chunks ahead of the read matmuls so the PE never
        # stalls on the cross-engine psum->sbuf evacuation roundtrip
        wTs = []
        for it in range(MC + LAG):
            if it < MC:
                mc = it
                if mc % GRP == 0 and mc >= PRE * GRP:
                    g = mc // GRP
                    nc.sync.dma_start(
                        mgrp[g][:], memR[g * GRP:(g + 1) * GRP]
                        .rearrange("c p d -> p c d"))
                tp = ps_tr.tile([P, B_CORE], F16, tag="wtr")
                for bt in range(NBT):
                    nc.tensor.transpose(
                        tp[:, bt * P:(bt + 1) * P],
                        w[bt][:, mc * P:(mc + 1) * P], ident[:])
                wT = wTt.tile([P, B_CORE], F16, tag="wT", name=f"wT{mc}")
                wTs.append(wT)
                if mc % 2 == 0:
                    nc.vector.tensor_copy(wT[:], tp[:])
                else:
                    nc.scalar.copy(wT[:], tp[:])
            if it >= LAG:
                mc = it - LAG
                gt = mgrp[mc // GRP]
                sl = mc % GRP
                for bt in range(NBT):
                    for dh in range(2):
                        nc.tensor.matmul(
                            mv_ps[bt][dh][:], wTs[mc][:, bt * P:(bt + 1) * P],
                            gt[:, sl, dh * 512:(dh + 1) * 512],
                            start=(mc == 0), stop=(mc == MC - 1))

        # ---- mv evac (fp16) + transpose to mvT [P, dc, 256] ----
        mv_sb = [small.tile([P, D], F16, tag=f"mvsb{bt}", name=f"mvsb{bt}")
                 for bt in range(NBT)]
        for bt in range(NBT):
            for dh in range(2):
                nc.scalar.copy(mv_sb[bt][:, dh * 512:(dh + 1) * 512],
                               mv_ps[bt][dh][:])
        ps_mv.release()
        mvT = small.tile([P, DC, B_CORE], F16, tag="mvT")
        for dc in range(DC):
            tp = ps_tr.tile([P, B_CORE], F16, tag="wtr")
            for bt in range(NBT):
                nc.tensor.transpose(
                    tp[:, bt * P:(bt + 1) * P],
                    mv_sb[bt][:, dc * P:(dc + 1) * P], ident[:])
            if dc % 2 == 0:
                nc.vector.tensor_copy(mvT[:, dc], tp[:])
            else:
                nc.scalar.copy(mvT[:, dc], tp[:])

        # ---- MLP1: h[bt] [P(b), H] = mvT-blocks^T @ W1-chunks (bias later) --
        hsb = [small.tile([P, H], F16, tag=f"h{bt}", name=f"h{bt}")
               for bt in range(NBT)]
        ps_h = tc.alloc_tile_pool(name="ps_h", bufs=1, space="PSUM")
        hps = [ps_h.tile([P, 512], F32, tag=f"hp{hb}", name=f"hp{hb}")
               for hb in range(HB)]
        for bt in range(NBT):
            for dc in range(DC):
                for hb in range(HB):
                    nc.tensor.matmul(
                        hps[hb][:], mvT[:, dc, bt * P:(bt + 1) * P],
                        w1t[dc][:, hb * 512:(hb + 1) * 512],
                        start=(dc == 0), stop=(dc == DC - 1))
            for hb in range(HB):
                nc.scalar.copy(hsb[bt][:, hb * 512:(hb + 1) * 512],
                               hps[hb][:])
        ps_h.release()

        # ---- hT transposes; relu + b1 fused into the per-partition evac ----
        hT = small.tile([P, HC, B_CORE], F16, tag="hT")
        for hc in range(HC):
            tp = ps_tr.tile([P, B_CORE], F16, tag="wtr")
            for bt in range(NBT):
                nc.tensor.transpose(
                    tp[:, bt * P:(bt + 1) * P],
                    hsb[bt][:, hc * P:(hc + 1) * P], ident[:])
            nc.scalar.activation(
                hT[:, hc], tp[:], AF.Relu, bias=b1t[:, hc:hc + 1])

        # ---- MLP2: out[bt] = hT-blocks^T @ W2 + b2 ----
        ps_o = tc.alloc_tile_pool(name="ps_o", bufs=1, space="PSUM")
        osb = [small.tile([P, OUT], F32, tag=f"osb{bt}", name=f"osb{bt}")
               for bt in range(NBT)]
        ops = [[ps_o.tile([P, NW], F32, tag=f"op{bt}_{oh}",
                          name=f"op{bt}_{oh}")
                for oh in range(NOH)] for bt in range(NBT)]
        for bt in range(NBT):
            for oh in range(NOH):
                nc.tensor.matmul(
                    ops[bt][oh][:], ones1[:], b2t[:, oh * NW:(oh + 1) * NW],
                    start=True, stop=False)
        for kc in range(HC):
            for bt in range(NBT):
                for oh in range(NOH):
                    nc.tensor.matmul(
                        ops[bt][oh][:], hT[:, kc, bt * P:(bt + 1) * P],
                        w2t[kc // 2][:, kc % 2, oh * NW:(oh + 1) * NW],
                        start=False, stop=(kc == HC - 1))
        for bt in range(NBT):
            for oh in range(NOH):
                nc.scalar.copy(osb[bt][:, oh * NW:(oh + 1) * NW],
                               ops[bt][oh][:])
                nc.gpsimd.dma_start(
                    out_d[bt, :, oh * NW:(oh + 1) * NW],
                    osb[bt][:, oh * NW:(oh + 1) * NW])
        ps_o.release()
        ps_tr.release()
        w2s.release()
        mring.release()
        wTt.release()
        w1p.release()
        wpool.release()
        small.release()

    nc.compile()
    return nc


_CACHED = None


def _prep(inputs):
    x = np.ascontiguousarray(inputs["encoder_output"], dtype=np.float32)
    mem = np.ascontiguousarray(inputs["memory_set"], dtype=np.float32)
    W1 = np.ascontiguousarray(inputs["W1"], dtype=np.float32)
    b1 = np.ascontiguousarray(inputs["b1"], dtype=np.float32)
    W2 = np.ascontiguousarray(inputs["W2"], dtype=np.float32)
    b2 = np.ascontiguousarray(inputs["b2"], dtype=np.float32)

    inv_nx = 1.0 / np.sqrt((x * x).sum(1) + _EPS)
    inv_ny = 1.0 / np.sqrt((mem * mem).sum(1) + _EPS)
    xh = (x * inv_nx[:, None]).astype(np.float16)
    memT_hat = np.ascontiguousarray(
        (mem.T * inv_ny[None, :]).astype(np.float16).reshape(DC, P, M))
    memR_v = np.ascontiguousarray(mem.astype(np.float16).reshape(MC, P, D))
    w1_blk = np.ascontiguousarray(W1.astype(np.float16).reshape(DC, P, H))
    w2_blk = np.ascontiguousarray(W2.astype(np.float16).reshape(HC, P, OUT))
    b1_tiles = np.ascontiguousarray(b1.reshape(HC, P).T.astype(np.float32))
    b2_row = np.ascontiguousarray(b2.reshape(1, OUT).astype(np.float16))

    shared = {
        "memT": memT_hat, "memR": memR_v, "w1c": w1_blk,
        "w2c": w2_blk, "b1_t": b1_tiles, "b2_r": b2_row,
    }
    in_maps = []
    for c in range(8):
        xc = xh[c * B_CORE:(c + 1) * B_CORE]          # [256, 1024]
        xhT = np.ascontiguousarray(
            xc.T.reshape(DC, P, B_CORE).transpose(1, 0, 2)
            .reshape(P, DC * B_CORE))
        in_maps.append({"xhT": xhT, **shared})
    return in_maps


def kernel(**inputs) -> np.ndarray:
    global _CACHED
    if _CACHED is None:
        _CACHED = build()
    nc = _CACHED
    in_maps = _prep(inputs)
    res = run_bass_kernel_spmd(nc, in_maps, core_ids=list(range(8)))
    return np.concatenate(
        [r["out"].reshape(B_CORE, OUT) for r in res.results], axis=0)


# revision 30
# speedup vs baseline: 1.0660x; 1.0050x over previous
"""Trainium2 Bass kernel for nn_BaselineMemory (sparse attention memory read + MLP).

Data-parallel over batch: each of 8 NeuronCores handles 256 of 2048 rows.
fp16 end-to-end (fp32 PSUM/accum). Host pre-normalizes x and mem^T.
Pipeline per core:
  dist matmul z = x_hat @ y_hat^T (fp16, PE) -> z fp16 + block sums/maxes
  -> sparsemax tau via 3 Newton rounds from a Gaussian-moment init
     (S(tau) = ACT relu head + DVE tail; support count k on DVE is_ge;
      tau += (S-1)/k) -> w materialization pass
  -> w^T transposes (PE, fp16) -> memory read mv^T (fp16, PE)
  -> MLP1 (W1 natural layout, relu+b1 fused into post-transpose evac)
  -> MLP2 (+b2 via rank-1 matmul) -> fp32 out.
DMA: memT stream + memR ring + outputs on the Sync queue; W1/W2 on the
GpSimd queue so they transfer during the sparsemax window.
"""
import sys

if "/opt/trn_rl_repo" not in sys.path:
    sys.path.insert(0, "/opt/trn_rl_repo")

import numpy as np

import concourse.bass as bass  # noqa: F401
import concourse.tile as tile
from concourse import bacc, mybir
from concourse.bass_utils import run_bass_kernel_spmd
from concourse.masks import make_identity

P = 128
B_CORE = 256          # batch rows per core
NBT = B_CORE // P     # 2 b-tiles
D = 1024
DC = D // P           # 8 d-chunks
M = 8192
MB = M // 512         # 16 dist m-blocks per bt
MC = M // P           # 64 m-chunks for read
H = 2048
HC = H // P           # 16 h-chunks
HB = H // 512         # 4 mlp1 col-blocks
OUT = 1000
NOH = 2               # out halves of 500
NW = OUT // NOH

N_ROUNDS = 2          # quasi-Newton iterations on tau
MA = 5120             # S-pass head handled by ACT; tail [MA, M) on DVE
TAIL = M - MA
T0_SIG = 2.25 / 32.0  # init: tau0 = mean + 2.25*sigma, sigma = 1/sqrt(d)
INV_S2 = 22.627417    # 1/(sigma*sqrt(2)) = 32/sqrt(2)
KHALF = 4096.0        # m/2 for the erfc slope model
CLIP = 1.0 / 16.0     # step clip (2*sigma)
CAP_OFF = 1e-4
RING = 24             # memR ring slabs resident
W2BUF = 8
LAG = 12              # read matmuls trail w^T transposes by LAG chunks

F32 = mybir.dt.float32
F16 = mybir.dt.float16
AF = mybir.ActivationFunctionType
ALU = mybir.AluOpType
AX = mybir.AxisListType

_EPS = 1e-6


def build():
    nc = bacc.Bacc("TRN2", target_bir_lowering=False, debug=False)

    xhT_d = nc.dram_tensor("xhT", [P, DC * B_CORE], F16, kind="ExternalInput")
    memT = nc.dram_tensor("memT", [DC, P, M], F16, kind="ExternalInput")
    memR = nc.dram_tensor("memR", [MC, P, D], F16, kind="ExternalInput")
    w1c = nc.dram_tensor("w1c", [DC, P, H], F16, kind="ExternalInput")
    w2c = nc.dram_tensor("w2c", [HC, P, OUT], F16, kind="ExternalInput")
    b1_t = nc.dram_tensor("b1_t", [P, HC], F32, kind="ExternalInput")
    b2_r = nc.dram_tensor("b2_r", [1, OUT], F16, kind="ExternalInput")
    out_d = nc.dram_tensor("out", [NBT, P, OUT], F32, kind="ExternalOutput")

    with tile.TileContext(nc) as tc:
        small = tc.alloc_tile_pool(name="small", bufs=1)
        wpool = tc.alloc_tile_pool(name="wpool", bufs=1)
        w1p = tc.alloc_tile_pool(name="w1p", bufs=1)

        ident = small.tile([P, P], F16, tag="ident")
        make_identity(nc, ident[:])
        ones1 = small.tile([1, P], F16, tag="ones1")
        nc.vector.memset(ones1[:], 1.0)
        b1t = small.tile([P, HC], F32, tag="b1")
        nc.gpsimd.dma_start(b1t[:], b1_t[:])
        b2t = small.tile([1, OUT], F16, tag="b2")
        nc.gpsimd.dma_start(b2t[:], b2_r[:])
        xh = small.tile([P, DC * B_CORE], F16, tag="xh")
        nc.sync.dma_start(xh[:], xhT_d[:])

        w = [wpool.tile([P, M], F16, tag=f"w{bt}", name=f"w{bt}")
             for bt in range(NBT)]
        w1t = [w1p.tile([P, H], F16, tag=f"w1_{dc}", name=f"w1_{dc}")
               for dc in range(DC)]

        st = {}
        for bt in range(NBT):
            d = {}
            d["mx"] = small.tile([P, MB], F32, tag=f"mx{bt}", name=f"mx{bt}")
            d["zsum"] = small.tile([P, MB], F32, tag=f"zs{bt}", name=f"zs{bt}")
            for nm in ["rm", "cap", "zsr", "mu", "sact", "sactB", "gacc",
                       "targ", "erf", "kg", "rk", "sv", "step", "stepc"]:
                d[nm] = small.tile([P, 1], F32, tag=f"{nm}{bt}", name=f"{nm}{bt}")
            d["tau"] = [small.tile([P, 1], F32, tag=f"tau{bt}_{r}",
                                   name=f"tau{bt}_{r}")
                        for r in range(N_ROUNDS + 1)]
            d["ntau"] = [small.tile([P, 1], F32, tag=f"ntau{bt}_{r}",
                                    name=f"ntau{bt}_{r}")
                         for r in range(N_ROUNDS + 1)]
            st[bt] = d

        # ---- persistent row tiles (released after w materialization) ----
        zpool = tc.alloc_tile_pool(name="zpool", bufs=1)
        z = [zpool.tile([P, M], F16, tag=f"z{bt}", name=f"z{bt}")
             for bt in range(NBT)]
        scr = [zpool.tile([P, TAIL], F16, tag=f"scr{bt}", name=f"scr{bt}")
               for bt in range(NBT)]
        tr1 = [zpool.tile([P, TAIL // 2], F16, tag=f"tr1{bt}", name=f"tr1{bt}")
               for bt in range(NBT)]
        tr2 = [zpool.tile([P, TAIL // 4], F16, tag=f"tr2{bt}", name=f"tr2{bt}")
               for bt in range(NBT)]
        tr3 = [zpool.tile([P, TAIL // 8], F16, tag=f"tr3{bt}", name=f"tr3{bt}")
               for bt in range(NBT)]

        # ---- PE warmup burst: ramp the clock while the first DMAs land ----
        junk = small.tile([P, 512], F16, tag="junk")
        nc.vector.memset(junk[:], 1.0)

        # ---- dist: z[bt] [P, M] fp16 + block sums/maxes ----
        mstream = tc.alloc_tile_pool(name="mstream", bufs=2)
        ps_wu = tc.alloc_tile_pool(name="ps_wu", bufs=2, space="PSUM")
        for i in range(10):
            wup = ps_wu.tile([P, 512], F32, tag="wu")
            nc.tensor.matmul(wup[:], ident[:], junk[:], start=True, stop=True)
        ps_wu.release()
        ps_dist = tc.alloc_tile_pool(name="ps_dist", bufs=4, space="PSUM")
        ps_warm = tc.alloc_tile_pool(name="ps_warm", bufs=2, space="PSUM")
        NSTAT = 12            # half-blocks used for the tau0 stats

        def emit_init():
            for bt in range(NBT):
                d = st[bt]
                nc.vector.reduce_sum(d["zsr"][:], d["zsum"][:, 0:NSTAT],
                                     axis=AX.X)
                nc.vector.reduce_max(d["rm"][:], d["mx"][:, 0:NSTAT],
                                     axis=AX.X)
                nc.vector.tensor_scalar_add(d["cap"][:], d["rm"][:], -CAP_OFF)
                nc.vector.tensor_scalar_mul(d["mu"][:], d["zsr"][:],
                                            1.0 / (NSTAT * 512))
                nc.vector.tensor_scalar_add(d["step"][:], d["mu"][:], T0_SIG)
                nc.vector.tensor_tensor(
                    d["tau"][0][:], d["step"][:], d["cap"][:], ALU.min)
                nc.vector.tensor_scalar_mul(d["ntau"][0][:], d["tau"][0][:],
                                            -1.0)

        def emit_round_head(r):
            for bt in range(NBT):
                d = st[bt]
                nc.vector.tensor_scalar(
                    out=d["targ"][:], in0=d["tau"][r][:],
                    scalar1=d["mu"][:, 0:1],
                    scalar2=INV_S2, op0=ALU.subtract, op1=ALU.mult)
            for bt in range(NBT):
                d = st[bt]
                nc.scalar.activation(
                    w[bt][:, 0:MA], z[bt][:, 0:MA], AF.Relu,
                    bias=d["ntau"][r][:, 0:1], accum_out=d["sact"][:])
                nc.scalar.activation(d["erf"][:], d["targ"][:], AF.Erf)
            for bt in range(NBT):
                d = st[bt]
                nc.vector.tensor_scalar(
                    out=d["kg"][:], in0=d["erf"][:], scalar1=-KHALF,
                    scalar2=KHALF, op0=ALU.mult, op1=ALU.add)
                nc.vector.tensor_scalar_max(d["kg"][:], d["kg"][:], 1.0)
                nc.vector.reciprocal(d["rk"][:], d["kg"][:])

        def emit_round_tail(r):
            for bt in range(NBT):
                d = st[bt]
                tau_s = d["tau"][r][:, 0:1]
                nc.vector.tensor_scalar(
                    out=scr[bt][:], in0=z[bt][:, MA:M],
                    scalar1=tau_s, scalar2=None, op0=ALU.subtract)
                nc.vector.tensor_scalar(
                    out=w[bt][:, MA:M], in0=scr[bt][:],
                    scalar1=0.0, scalar2=None, op0=ALU.max)
                h2 = TAIL // 2
                h4 = TAIL // 4
                nc.vector.tensor_add(
                    tr1[bt][:], w[bt][:, MA:MA + h2], w[bt][:, MA + h2:M])
                nc.vector.tensor_add(
                    tr2[bt][:], tr1[bt][:, 0:h4], tr1[bt][:, h4:h2])
                h8 = TAIL // 8
                nc.vector.tensor_add(
                    tr3[bt][:], tr2[bt][:, 0:h8], tr2[bt][:, h8:h4])
                nc.vector.tensor_scalar(
                    out=tr3[bt][:], in0=tr3[bt][:], scalar1=0.0, scalar2=None,
                    op0=ALU.add, op1=ALU.add, accum_out=d["gacc"][:])
                nc.vector.tensor_add(d["sv"][:], d["sact"][:], d["gacc"][:])
                nc.vector.tensor_scalar(
                    out=d["step"][:], in0=d["sv"][:], scalar1=-1.0,
                    scalar2=d["rk"][:, 0:1], op0=ALU.add, op1=ALU.mult)
                nc.vector.tensor_scalar(
                    out=d["stepc"][:], in0=d["step"][:], scalar1=CLIP,
                    scalar2=-CLIP, op0=ALU.min, op1=ALU.max)
                nc.vector.tensor_scalar(
                    out=d["tau"][r + 1][:], in0=d["stepc"][:],
                    scalar1=d["tau"][r][:, 0:1], scalar2=d["cap"][:, 0:1],
                    op0=ALU.add, op1=ALU.min)
                nc.vector.tensor_scalar_mul(
                    d["ntau"][r + 1][:], d["tau"][r + 1][:], -1.0)
            # keep the PE clock from dropping to the lowest p-state
            for i in range(4):
                wp = ps_warm.tile([P, 512], F32, tag="warm")
                nc.tensor.matmul(wp[:], ident[:],
                                 scr[i % NBT][:, 0:512], start=True, stop=True)

        for blk in range(MB // 2):
            mt = mstream.tile([P, DC, 1024], F16, tag="memT")
            if blk == 0:
                # split block 0 by m-halves: the first matmuls start as soon
                # as the first half lands
                for mh in range(2):
                    nc.sync.dma_start(
                        mt[:, :, mh * 512:(mh + 1) * 512],
                        memT[:, :, mh * 512:(mh + 1) * 512]
                        .rearrange("d p m -> p d m"))
            else:
                for dq in range(2):
                    nc.sync.dma_start(
                        mt[:, dq * 4:(dq + 1) * 4],
                        memT[dq * 4:(dq + 1) * 4,
                             :, blk * 1024:(blk + 1) * 1024]
                        .rearrange("d p m -> p d m"))
            for mh in range(2):
                mb = blk * 2 + mh
                for bt in range(NBT):
                    zp = ps_dist.tile([P, 512], F32, tag="zp")
                    for dc in range(DC):
                        nc.tensor.matmul(
                            zp[:],
                            xh[:, dc * B_CORE + bt * P:
                               dc * B_CORE + (bt + 1) * P],
                            mt[:, dc, mh * 512:(mh + 1) * 512],
                            start=(dc == 0), stop=(dc == DC - 1))
                    if mb < NSTAT:
                        nc.scalar.activation(
                            z[bt][:, mb * 512:(mb + 1) * 512], zp[:], AF.Copy,
                            accum_out=st[bt]["zsum"][:, mb:mb + 1])
                        nc.vector.reduce_max(
                            st[bt]["mx"][:, mb:mb + 1],
                            z[bt][:, mb * 512:(mb + 1) * 512], axis=AX.X)
                    else:
                        # late blocks evacuate on DVE so the ACT queue is
                        # free for round-0 heads during the dist tail
                        nc.vector.tensor_copy(
                            z[bt][:, mb * 512:(mb + 1) * 512], zp[:])
            if blk == (NSTAT // 2) - 1:
                emit_init()
                emit_round_head(0)



        # ---- remaining rounds (round-0 head already ran inside dist) ----
        emit_round_tail(0)
        for r in range(1, N_ROUNDS):
            emit_round_head(r)
            emit_round_tail(r)

        # ---- final w materialization at converged tau (split into column
        # halves so the first w^T transposes can start sooner) ----
        MAH = MA // 2
        for bt in range(NBT):
            d = st[bt]
            nf = d["ntau"][N_ROUNDS]
            nc.scalar.activation(
                w[bt][:, 0:MAH], z[bt][:, 0:MAH], AF.Relu, bias=nf[:, 0:1])
        for bt in range(NBT):
            d = st[bt]
            nf = d["ntau"][N_ROUNDS]
            nc.scalar.activation(
                w[bt][:, MAH:MA], z[bt][:, MAH:MA], AF.Relu, bias=nf[:, 0:1])
        for bt in range(NBT):
            d = st[bt]
            tau_s = d["tau"][N_ROUNDS][:, 0:1]
            nc.vector.tensor_scalar(
                out=scr[bt][:], in0=z[bt][:, MA:M],
                scalar1=tau_s, scalar2=None, op0=ALU.subtract)
            nc.vector.tensor_scalar(
                out=w[bt][:, MA:M], in0=scr[bt][:],
                scalar1=0.0, scalar2=None, op0=ALU.max)
        for i in range(4):
            wp = ps_warm.tile([P, 512], F32, tag="warm")
            nc.tensor.matmul(wp[:], ident[:], junk[:], start=True, stop=True)
        ps_warm.release()
        ps_dist.release()
        mstream.release()
        zpool.release()

        # ---- w^T transposes + memory read: mv[bt] = w[bt] @ memR ----
        wTt = tc.alloc_tile_pool(name="wTt", bufs=16)
        mring = tc.alloc_tile_pool(name="mring", bufs=3)
        w2s = tc.alloc_tile_pool(name="w2s", bufs=W2BUF)
        ps_tr = tc.alloc_tile_pool(name="ps_tr", bufs=4, space="PSUM")
        ps_mv = tc.alloc_tile_pool(name="ps_mv", bufs=1, space="PSUM")

        # memR ring: 8-slab groups (big DMAs amortize the trigger cost).
        # Prefill transfers run during the sparsemax window.
        GRP = 8
        NGRP = MC // GRP
        PRE = 3
        mgrp = []
        for g in range(NGRP):
            gt = mring.tile([P, GRP, D], F16, tag="memR", name=f"memR{g}")
            mgrp.append(gt)
            if g < PRE:
                nc.sync.dma_start(
                    gt[:], memR[g * GRP:(g + 1) * GRP]
                    .rearrange("c p d -> p c d"))
        # weights after the prefill: they arrive during the sparsemax window
        for dc in range(DC):
            nc.sync.dma_start(w1t[dc][:], w1c[dc])
        w2t = [w2s.tile([P, 2, OUT], F16, tag="w2", name=f"w2p_{g}")
               for g in range(HC // 2)]
        for g in range(HC // 2):
            nc.sync.dma_start(
                w2t[g][:],
                w2c[g * 2:(g + 1) * 2].rearrange("c p o -> p c o"))

        mv_ps = [[ps_mv.tile([P, 512], F32, tag=f"mv{bt}_{dh}",
                             name=f"mv{bt}_{dh}")
                  for dh in range(2)] for bt in range(NBT)]
        # transposes run LAG chunks ahead of the read matmuls so the PE never
        # stalls on the cross-engine psum->sbuf evacuation roundtrip
        wTs = []
        for it in range(MC + LAG):
            if it < MC:
                mc = it
                if mc % GRP == 0 and mc >= PRE * GRP:
                    g = mc // GRP
                    nc.sync.dma_start(
                        mgrp[g][:], memR[g * GRP:(g + 1) * GRP]
                        .rearrange("c p d -> p c d"))
                tp = ps_tr.tile([P, B_CORE], F16, tag="wtr")
                for bt in range(NBT):
                    nc.tensor.transpose(
                        tp[:, bt * P:(bt + 1) * P],
                        w[bt][:, mc * P:(mc + 1) * P], ident[:])
                wT = wTt.tile([P, B_CORE], F16, tag="wT", name=f"wT{mc}")
                wTs.append(wT)
                if mc % 2 == 0:
                    nc.vector.tensor_copy(wT[:], tp[:])
                else:
                    nc.scalar.copy(wT[:], tp[:])
            if it >= LAG:
                mc = it - LAG
                gt = mgrp[mc // GRP]
                sl = mc % GRP
                for bt in range(NBT):
                    for dh in range(2):
                        nc.tensor.matmul(
                            mv_ps[bt][dh][:], wTs[mc][:, bt * P:(bt + 1) * P],
                            gt[:, sl, dh * 512:(dh + 1) * 512],
                            start=(mc == 0), stop=(mc == MC - 1))

        # ---- mv evac (fp16) + transpose to mvT [P, dc, 256] ----
        mv_sb = [small.tile([P, D], F16, tag=f"mvsb{bt}", name=f"mvsb{bt}")
                 for bt in range(NBT)]
        for bt in range(NBT):
            for dh in range(2):
                nc.scalar.copy(mv_sb[bt][:, dh * 512:(dh + 1) * 512],
                               mv_ps[bt][dh][:])
        ps_mv.release()
        mvT = small.tile([P, DC, B_CORE], F16, tag="mvT")
        for dc in range(DC):
            tp = ps_tr.tile([P, B_CORE], F16, tag="wtr")
            for bt in range(NBT):
                nc.tensor.transpose(
                    tp[:, bt * P:(bt + 1) * P],
                    mv_sb[bt][:, dc * P:(dc + 1) * P], ident[:])
            if dc % 2 == 0:
                nc.vector.tensor_copy(mvT[:, dc], tp[:])
            else:
                nc.scalar.copy(mvT[:, dc], tp[:])

        # ---- MLP1: h[bt] [P(b), H] = mvT-blocks^T @ W1-chunks (bias later) --
        hsb = [small.tile([P, H], F16, tag=f"h{bt}", name=f"h{bt}")
               for bt in range(NBT)]
        ps_h = tc.alloc_tile_pool(name="ps_h", bufs=1, space="PSUM")
        hps = [ps_h.tile([P, 512], F32, tag=f"hp{hb}", name=f"hp{hb}")
               for hb in range(HB)]
        for bt in range(NBT):
            for dc in range(DC):
                for hb in range(HB):
                    nc.tensor.matmul(
                        hps[hb][:], mvT[:, dc, bt * P:(bt + 1) * P],
                        w1t[dc][:, hb * 512:(hb + 1) * 512],
                        start=(dc == 0), stop=(dc == DC - 1))
            for hb in range(HB):
                nc.scalar.copy(hsb[bt][:, hb * 512:(hb + 1) * 512],
                               hps[hb][:])
        ps_h.release()

        # ---- hT transposes; relu + b1 fused into the per-partition evac ----
        hT = small.tile([P, HC, B_CORE], F16, tag="hT")
        for hc in range(HC):
            tp = ps_tr.tile([P, B_CORE], F16, tag="wtr")
            for bt in range(NBT):
                nc.tensor.transpose(
                    tp[:, bt * P:(bt + 1) * P],
                    hsb[bt][:, hc * P:(hc + 1) * P], ident[:])
            nc.scalar.activation(
                hT[:, hc], tp[:], AF.Relu, bias=b1t[:, hc:hc + 1])

        # ---- MLP2: out[bt] = hT-blocks^T @ W2 + b2 ----
        ps_o = tc.alloc_tile_pool(name="ps_o", bufs=1, space="PSUM")
        osb = [small.tile([P, OUT], F32, tag=f"osb{bt}", name=f"osb{bt}")
               for bt in range(NBT)]
        ops = [[ps_o.tile([P, NW], F32, tag=f"op{bt}_{oh}",
                          name=f"op{bt}_{oh}")
                for oh in range(NOH)] for bt in range(NBT)]
        for bt in range(NBT):
            for oh in range(NOH):
                nc.tensor.matmul(
                    ops[bt][oh][:], ones1[:], b2t[:, oh * NW:(oh + 1) * NW],
                    start=True, stop=False)
        for kc in range(HC):
            for bt in range(NBT):
                for oh in range(NOH):
                    nc.tensor.matmul(
                        ops[bt][oh][:], hT[:, kc, bt * P:(bt + 1) * P],
                        w2t[kc // 2][:, kc % 2, oh * NW:(oh + 1) * NW],
                        start=False, stop=(kc == HC - 1))
        for bt in range(NBT):
            for oh in range(NOH):
                nc.scalar.copy(osb[bt][:, oh * NW:(oh + 1) * NW],
                               ops[bt][oh][:])
                nc.gpsimd.dma_start(
                    out_d[bt, :, oh * NW:(oh + 1) * NW],
                    osb[bt][:, oh * NW:(oh + 1) * NW])
        ps_o.release()
        ps_tr.release()
        w2s.release()
        mring.release()
        wTt.release()
        w1p.release()
        wpool.release()
        small.release()

    nc.compile()
    return nc


_CACHED = None


def _prep(inputs):
    x = np.ascontiguousarray(inputs["encoder_output"], dtype=np.float32)
    mem = np.ascontiguousarray(inputs["memory_set"], dtype=np.float32)
    W1 = np.ascontiguousarray(inputs["W1"], dtype=np.float32)
    b1 = np.ascontiguousarray(inputs["b1"], dtype=np.float32)
    W2 = np.ascontiguousarray(inputs["W2"], dtype=np.float32)
    b2 = np.ascontiguousarray(inputs["b2"], dtype=np.float32)

    inv_nx = 1.0 / np.sqrt((x * x).sum(1) + _EPS)
    inv_ny = 1.0 / np.sqrt((mem * mem).sum(1) + _EPS)
    xh = (x * inv_nx[:, None]).astype(np.float16)
    memT_hat = np.ascontiguousarray(
        (mem.T * inv_ny[None, :]).astype(np.float16).reshape(DC, P, M))
    memR_v = np.ascontiguousarray(mem.astype(np.float16).reshape(MC, P, D))
    w1_blk = np.ascontiguousarray(W1.astype(np.float16).reshape(DC, P, H))
    w2_blk = np.ascontiguousarray(W2.astype(np.float16).reshape(HC, P, OUT))
    b1_tiles = np.ascontiguousarray(b1.reshape(HC, P).T.astype(np.float32))
    b2_row = np.ascontiguousarray(b2.reshape(1, OUT).astype(np.float16))

    shared = {
        "memT": memT_hat, "memR": memR_v, "w1c": w1_blk,
        "w2c": w2_blk, "b1_t": b1_tiles, "b2_r": b2_row,
    }
    in_maps = []
    for c in range(8):
        xc = xh[c * B_CORE:(c + 1) * B_CORE]          # [256, 1024]
        xhT = np.ascontiguousarray(
            xc.T.reshape(DC, P, B_CORE).transpose(1, 0, 2)
            .reshape(P, DC * B_CORE))
        in_maps.append({"xhT": xhT, **shared})
    return in_maps


def kernel(**inputs) -> np.ndarray:
    global _CACHED
    if _CACHED is None:
        _CACHED = build()
    nc = _CACHED
    in_maps = _prep(inputs)
    res = run_bass_kernel_spmd(nc, in_maps, core_ids=list(range(8)))
    return np.concatenate(
        [r["out"].reshape(B_CORE, OUT) for r in res.results], axis=0)
